# revision 1
# baseline (speedup 1.0000x reference)
"""Trainium2 Bass kernel for CustomPositionsPiecewiseConv2d.

Math: for knots positions=[-1,-.5,0,.5,1] and inputs x in [0,1], the active
interpolation coefficients are
    c2 = relu(1-2v),  c4 = max(relu(2v-1), T),  c3 = 1 - c2 - c4
with T = 1[v >= theta] the isclose(v,1) mask.  Since c2+c3+c4 == 1 exactly
(everywhere, including the zero-padding border), the c3 plane folds away:
    out = C2 (x) (W2-W3) + C4 (x) (W4-W3) + sum_ck W3[o,c,k] + bias
Each plane is an elementwise function of v and v is shifted/padded copies of x,
so planes are computed once per padded image and the 3x3 im2col becomes 9
shifted access-pattern reads feeding PSUM-accumulated matmuls.

Modes:
  float32       exact, 4 cyc/row on PE
  float32r      tf32-rounded operands, 1 cyc/row (err ~1.6e-4)
  float32r_split  hi/lo tf32 split of coeffs+weights; per tap one K=128 matmul
                  [c2h,c4h,c2l,c4l]x[W2h,W4h,W2h,W4h] plus one K=64 matmul
                  [c2h,c4h]x[W2l,W4l]; drops only (lo x lo) terms ~2^-24.

Sharding: data-parallel over batch, 2 images per core on 8 cores.
"""

import numpy as np

B, C, H, W = 16, 32, 64, 64
O, P, KH, KW = 128, 5, 3, 3
NCORES = 8
IPC = B // NCORES            # images per core
HP, WP = H + 2, W + 2        # padded image (pad=1)
RT = 8                       # output rows per L-tile
NT = H // RT                 # L-tiles per image
K2 = KH * KW
L = H * W
ATOL = 1e-5
RTOL = 1e-5

MODE = "bf16_split"          # float32 | float32r | bf16_split


# ---------------------------------------------------------------- host math


def _isclose_np(a, b):
    return np.abs(a - b) <= np.float32(ATOL) + np.float32(RTOL) * np.abs(b)


def _reference_np(x, weights, bias, positions):
    """Direct numpy port of the reference (fallback path)."""
    EPS = 1e-6
    Bn, Cn, Hn, Wn = x.shape
    On, _, Pn, KHn, KWn = weights.shape
    xp = np.pad(x, ((0, 0), (0, 0), (1, 1), (1, 1)))
    cols = [
        xp[:, :, i : i + Hn, j : j + Wn] for i in range(KHn) for j in range(KWn)
    ]
    pat = np.stack(cols, axis=2)
    v = pat.reshape(Bn, Cn, KHn * KWn, Hn * Wn).astype(np.float32)

    left, right = positions[:-1], positions[1:]
    denom = right - left
    denom = np.where(denom == 0, np.float32(EPS), denom)
    varc = (1.0 / denom).astype(np.float32)
    const = (-left * varc).astype(np.float32)

    m_first = _isclose_np(v, positions[0])
    m_last = _isclose_np(v, positions[-1])
    in_range = (~(m_first | m_last)) & (v >= positions[0]) & (v <= positions[-1])

    coeff = np.zeros(v.shape + (Pn,), np.float32)
    coeff[..., 0] += m_first.astype(np.float32)
    coeff[..., Pn - 1] += m_last.astype(np.float32)
    for p in range(Pn - 1):
        m = (in_range & (v >= positions[p]) & (v < positions[p + 1])).astype(
            np.float32
        )
        t = v * varc[p] + const[p]
        coeff[..., p] += m * (1.0 - t)
        coeff[..., p + 1] += m * t

    Wk = np.transpose(weights, (0, 1, 3, 4, 2)).reshape(On, Cn, KHn * KWn, Pn)
    ident = np.all(np.abs(Wk - 1.0) <= np.float32(ATOL + RTOL), axis=-1)
    Wk_eff = np.where(ident[..., None], np.float32(0.0), Wk)

    out = np.einsum("bcklp,ockp->bol", coeff, Wk_eff, optimize=True)
    out = out + np.einsum(
        "bckl,ock->bol", v, ident.astype(np.float32), optimize=True
    )
    out = out + bias[None, :, None]
    return out.reshape(Bn, On, Hn, Wn).astype(np.float32)


def _compute_theta():
    """Smallest fp32 v such that fp32(1-v) <= fp32(ATOL + RTOL*1.0), matching
    the reference's m_last = isclose(v, 1.0) for v <= 1."""
    tau = np.float32(np.float32(ATOL) + np.float32(RTOL) * np.float32(1.0))
    th = np.float32(np.float32(1.0) - tau)
    while np.float32(np.float32(1.0) - np.nextafter(th, np.float32(0.0))) <= tau:
        th = np.nextafter(th, np.float32(0.0))
    while np.float32(np.float32(1.0) - th) > tau:
        th = np.nextafter(th, np.float32(2.0))
    return np.float32(th)


def _host_weights(weights, bias):
    """Fold c3 away.  Returns (wfold [2C, K2, O] f32 = [W2-W3; W4-W3],
    bias_eff [O] f32 = bias + sum_ck W3, ident_any)."""
    Wk = np.transpose(weights, (0, 1, 3, 4, 2)).reshape(O, C, K2, P)
    ident = np.all(np.abs(Wk - 1.0) <= np.float32(ATOL + RTOL), axis=-1)
    ident_any = bool(ident.any())
    Wk_eff = np.where(ident[..., None], np.float32(0.0), Wk)
    W2 = Wk_eff[:, :, :, 2].astype(np.float64)
    W3 = Wk_eff[:, :, :, 3].astype(np.float64)
    W4 = Wk_eff[:, :, :, 4].astype(np.float64)
    wfold = np.zeros((2 * C, K2, O), np.float32)
    wfold[0:C] = (W2 - W3).astype(np.float32).transpose(1, 2, 0)
    wfold[C : 2 * C] = (W4 - W3).astype(np.float32).transpose(1, 2, 0)
    bias_eff = (bias.astype(np.float64) + W3.sum(axis=(1, 2))).astype(np.float32)
    return np.ascontiguousarray(wfold), np.ascontiguousarray(bias_eff), ident_any


# ---------------------------------------------------------------- device IR


def _build_nc(theta, mode):
    import concourse.tile as tile
    from concourse import bacc, mybir

    f32 = mybir.dt.float32
    f32r = mybir.dt.float32r
    bf16 = mybir.dt.bfloat16
    Alu = mybir.AluOpType
    Act = mybir.ActivationFunctionType
    split = mode == "bf16_split"
    if mode == "float32":
        plane_dt = f32
    elif mode == "float32r":
        plane_dt = f32r
    else:
        plane_dt = bf16

    nc = bacc.Bacc("TRN2", target_bir_lowering=False, debug=False,
                   num_devices=NCORES)
    x_d = nc.dram_tensor("x", [IPC, C, H, W], f32, kind="ExternalInput").ap()
    w_d = nc.dram_tensor("wfold", [2 * C, K2, O], f32, kind="ExternalInput").ap()
    b_d = nc.dram_tensor("bias", [O, 1], f32, kind="ExternalInput").ap()
    o_d = nc.dram_tensor("out", [IPC, O, H, W], f32, kind="ExternalOutput").ap()

    with tile.TileContext(nc) as tc:
        with (
            tc.tile_pool(name="const", bufs=1) as constp,
            tc.tile_pool(name="scratch", bufs=1) as scrp,
            tc.tile_pool(name="plane", bufs=1) as planep,
            tc.tile_pool(name="ybuf", bufs=2) as ybufp,
            tc.tile_pool(name="psum", bufs=1, space="PSUM") as psump,
            tc.tile_pool(name="osb", bufs=4) as osbp,
        ):
            # ---- x loads first (phi critical path), weights after ----
            XF = scrp.tile([IPC * C, H, W], f32)      # flat x
            for i in range(IPC):
                nc.sync.dma_start(XF[i * C : (i + 1) * C], x_d[i])

            # pull the ACT table load off the critical path
            tiny = constp.tile([C, 1], f32)
            nc.gpsimd.memset(tiny[:], 0.0)
            nc.scalar.activation(tiny[:], tiny[:], Act.Relu, bias=0.0, scale=1.0)

            # PE warmup: dummy matmuls keep HAM at K=8/8 until the real
            # stream starts (otherwise the first ~5us of matmuls run at 1.2GHz)
            zb = constp.tile([128, 512], plane_dt)
            nc.gpsimd.memset(
                zb[:].bitcast(f32) if plane_dt == f32r else zb[:], 0.0
            )
            warm_ctr = [0]

            def warm(nmm, rhs=None):
                """Dummy matmuls (results never read). rhs gates when the
                batch can start, chaining PE busy-ness across the phi phase."""
                w = warm_ctr[0]
                warm_ctr[0] += 1
                pw = psump.tile(
                    [O, 512], f32, name=f"ps_warm{w}", tag=f"ps{w % 2}"
                )
                r = zb[:] if rhs is None else rhs
                kp = r.shape[0]
                for j in range(nmm):
                    nc.tensor.matmul(
                        pw[:], zb[0:kp, 0:128], r,
                        start=(j % 8 == 0), stop=(j % 8 == 7 or j == nmm - 1),
                    )

            warm(16)

            # ---- weights ----
            w_sb = constp.tile([2 * C, K2, O], f32)
            nc.sync.dma_start(w_sb[:], w_d[:])
            b_sb = constp.tile([O, 1], f32)
            nc.sync.dma_start(b_sb[:], b_d[:])
            if mode == "float32":
                w_hi = w_sb
            else:
                w_hi = constp.tile([2 * C, K2, O], plane_dt)
                nc.vector.tensor_copy(w_hi[:], w_sb[:])
            if split:
                w_lo = constp.tile([2 * C, K2, O], plane_dt)
                nc.vector.tensor_tensor(w_lo[:], w_sb[:], w_hi[:], Alu.subtract)
                # lhsT1 rows: [W2h, W4h, W2h, W4h] (hi coeffs then lo coeffs)
                w_rep = constp.tile([4 * C, K2, O], plane_dt)
                nc.sync.dma_start(w_rep[0 : 2 * C], w_hi[:])
                nc.sync.dma_start(w_rep[2 * C : 4 * C], w_hi[:])
                lhs1, lhs2 = w_rep, w_lo
            else:
                lhs1, lhs2 = w_hi, None

            # ---- coefficient planes ----
            # scratch on the same partitions as each image's plane slice
            # (engine ops require equal SBUF base partitions across operands)
            RF = scrp.tile([IPC * C, H, W], f32)
            CF = scrp.tile([IPC * C, H, W], f32)

            npl = 4 if split else 2
            # plane buffers, padded layout; group order:
            #   split: [c2h, c4h, c2l, c4l]   else: [c2, c4]
            PL = [
                planep.tile([IPC * C, HP, WP], plane_dt, name=f"PL{g}")
                for g in range(npl)
            ]
            # borders: c2-like planes = 1 at v=0, everything else = 0
            # (memset rejects f32r dests; same-size bitcast to f32 is a no-op)
            for g, pl in enumerate(PL):
                bv = 1.0 if g == 0 else 0.0
                for strip in (
                    pl[:, 0, :],
                    pl[:, HP - 1, :],
                    pl[:, 1 : HP - 1, 0],
                    pl[:, 1 : HP - 1, WP - 1],
                ):
                    nc.gpsimd.memset(
                        strip.bitcast(f32) if plane_dt == f32r else strip, bv
                    )

            def interior(pl):
                return pl[:, 1 : HP - 1, 1 : WP - 1]

            negone = constp.tile([IPC * C, 1], f32)
            nc.gpsimd.memset(negone[:], -1.0)

            def phi_chunk(r0, r1):
                """Coefficient planes for image rows [r0, r1), both images at
                once (64 partitions). Chunking lets the first GEMM tiles start
                while the rest of the planes are still being computed."""
                xf = XF[:, r0:r1, :]
                neg = negone[:]
                rf = RF[:, r0:r1, :]
                cf = CF[:, r0:r1, :]
                pls = [pl[:, 1 + r0 : 1 + r1, 1 : WP - 1] for pl in PL]
                if split:
                    # bf16 rounding absorbs the isclose(v,1) mask: for
                    # v >= 1-2^-9, relu(2v-1) rounds to exactly 1.0, and the
                    # lo-plane residual lands on hi weights scaled 2^-9.
                    c2h, c4h, c2l, c4l = pls
                    nc.scalar.activation(rf, xf, Act.Relu, bias=neg, scale=2.0)
                    nc.vector.tensor_copy(c4h, rf)
                    nc.vector.tensor_tensor(c4l, rf, c4h, Alu.subtract)
                    nc.scalar.activation(cf, xf, Act.Relu, bias=1.0, scale=-2.0)
                    nc.scalar.activation(c2h, cf, Act.Copy)
                    nc.vector.tensor_tensor(c2l, cf, c2h, Alu.subtract)
                else:
                    c2, c4 = pls
                    nc.vector.tensor_scalar(cf, xf, float(theta), None, Alu.is_ge)
                    nc.scalar.activation(rf, xf, Act.Relu, bias=neg, scale=2.0)
                    nc.vector.tensor_tensor(rf, rf, cf, Alu.max)
                    nc.vector.tensor_copy(c4, rf)
                    nc.scalar.activation(c2, xf, Act.Relu, bias=1.0, scale=-2.0)

            phi_chunk(0, H)
            # bridge PE busy-ness across the phi phase: each batch is gated
            # on a progressively later plane artifact (HAM re-throttles after
            # ~3.4us of PE idle, and a cold stream runs at 1.2GHz)
            nh = npl * C // 2
            order = [1, 3, 0, 2] if split else [1, 0]
            for g in order:
                warm(8, PL[g][0:nh, 0:RT, 0:W])

            # ---- per-image gather + GEMM ----
            # Tap-outer loop: one LDWEIGHTS feeds a run of back-to-back
            # matmuls (same stationary operand), so drain overlaps the next
            # fill and the per-MM cost stays ~N/2.4 instead of the isolated
            # latency. All 8 L-tiles of an image accumulate in 8 PSUM banks.
            # Tiles are issued in two groups gated on the two phi row-chunks
            # (Tile deps are byte-range granular), so the GEMM starts as soon
            # as the first chunk of planes is gathered into Y.
            def tap_loop(Y, pss, tiles, start, stop):
                for ki in range(K2):
                    kh, kw = divmod(ki, KW)
                    cols = slice(kw, kw + W)
                    last = ki == K2 - 1
                    for t in tiles:
                        rows = slice(t * RT + kh, t * RT + kh + RT)
                        nc.tensor.matmul(
                            pss[t][:], lhs1[:, ki, :], Y[:, rows, cols],
                            start=(start and ki == 0),
                            stop=(stop and last and not split),
                        )
                    if split:
                        for t in tiles:
                            rows = slice(t * RT + kh, t * RT + kh + RT)
                            nc.tensor.matmul(
                                pss[t][:], lhs2[:, ki, :],
                                Y[0 : 2 * C, rows, cols],
                                start=False, stop=(stop and last),
                            )

            for i in range(IPC):
                Y = ybufp.tile([npl * C, HP, WP], plane_dt, name="Y", tag="Y")
                s = slice(i * C, (i + 1) * C)
                for g, pl in enumerate(PL):
                    nc.sync.dma_start(Y[g * C : (g + 1) * C], pl[s])

                pss = [
                    psump.tile([O, RT * W], f32, name=f"ps{t}", tag=f"ps{t}")
                    for t in range(NT)
                ]
                tap_loop(Y, pss, list(range(NT)), start=True, stop=True)
                for t in range(NT):
                    osb = osbp.tile([O, RT * W], f32, name="osb")
                    if t % 2 == 0:
                        nc.scalar.activation(
                            osb[:], pss[t][:], Act.Identity, bias=b_sb[:, 0:1],
                            scale=1.0,
                        )
                    else:
                        nc.vector.tensor_scalar(
                            osb[:], pss[t][:], b_sb[:, 0:1], None, Alu.add
                        )
                    nc.sync.dma_start(
                        o_d[i, :, t * RT : (t + 1) * RT, :],
                        osb[:].rearrange("o (r w) -> o r w", r=RT),
                    )
    nc.compile()
    return nc


# ---------------------------------------------------------------- entry


def _prep(inputs):
    x = np.ascontiguousarray(np.asarray(inputs["x"], dtype=np.float32))
    weights = np.ascontiguousarray(np.asarray(inputs["weights"], dtype=np.float32))
    bias = np.ascontiguousarray(np.asarray(inputs["bias"], dtype=np.float32))
    positions = np.ascontiguousarray(
        np.asarray(inputs["positions"], dtype=np.float32)
    )
    return x, weights, bias, positions


def _fast_path_ok(x, positions):
    expect = np.linspace(-1.0, 1.0, P, dtype=np.float32)
    return (
        x.shape == (B, C, H, W)
        and positions.shape == (P,)
        and np.array_equal(positions, expect)
        and float(x.min()) >= 0.0
        and float(x.max()) <= 1.0
    )


def kernel(**inputs):
    x, weights, bias, positions = _prep(inputs)
    if not _fast_path_ok(x, positions):
        return _reference_np(x, weights, bias, positions)

    wfold, bias_eff, ident_any = _host_weights(weights, bias)
    if ident_any:
        # identity-shortcut weights present: needs the raw-v plane; use the
        # exact fallback rather than a rarely-exercised device path
        return _reference_np(x, weights, bias, positions)

    from concourse.bass_utils import run_bass_kernel_spmd

    nc = _build_nc(_compute_theta(), MODE)
    bias2d = np.ascontiguousarray(bias_eff.reshape(O, 1))
    in_maps = [
        {"x": np.ascontiguousarray(x[i * IPC : (i + 1) * IPC]),
         "wfold": wfold, "bias": bias2d}
        for i in range(NCORES)
    ]
    res = run_bass_kernel_spmd(nc, in_maps, core_ids=list(range(NCORES)))
    out = np.concatenate([res.results[i]["out"] for i in range(NCORES)], axis=0)
    return np.ascontiguousarray(out)


# ------------------------------------------------------------ dev utilities


def _run_sim(inputs):
    """CoreSim single-core run (images 0..IPC-1) for correctness debugging."""
    from concourse.bass_interp import CoreSim

    x, weights, bias, positions = _prep(inputs)
    assert _fast_path_ok(x, positions)
    wfold, bias_eff, ident_any = _host_weights(weights, bias)
    assert not ident_any
    nc = _build_nc(_compute_theta(), MODE)
    sim = CoreSim(nc)
    sim.tensor("x")[:] = x[:IPC]
    sim.tensor("wfold")[:] = wfold
    sim.tensor("bias")[:] = bias_eff.reshape(O, 1)
    sim.simulate()
    return np.array(sim.tensor("out"))



# revision 5
# speedup vs baseline: 1.5100x; 1.5100x over previous
"""Trainium2 Bass kernel for CustomPositionsPiecewiseConv2d.

Math: for knots [-1,-.5,0,.5,1] and x in [0,1] the active coefficients are
c2 = relu(1-2v), c4 = relu(2v-1), c3 = 1-c2-c4.  Folding c3 away and then
eliminating c2 via the identity c2 - c4 = 1 - 2v gives a TWO-plane GEMM:

    out = sum_ck  g(v) * A'[c,k,o]  +  v * B[c,k,o]  + bias_eff[o]
    g(v) = relu(v - 0.5)            (= c4/2)
    A'   = 2*(W2 + W4 - 2*W3)
    B    = -2*(W2 - W3)
    bias_eff = bias + sum_ck W2

The v-plane is just (padded, bf16-cast) x — no compute — and g needs one
activation op.  Both planes are 0 on the zero-padding border.

Tap packing: Y holds [v, g, v-shift-up-1row, g-shift-up-1row] across 128
partitions, so one K=128 matmul covers taps (0,kw) and (1,kw) at once.
Taps (2,0)/(2,1) run as two concurrent K=64 row-tiled matmuls (array rows
0-63 / 64-127); tap (2,2) is a lone K=64 matmul.  5 matmul slots per
output tile instead of 9 (theoretical floor 4.5).

Everything is bf16 on the PE (err ~3e-3 << the 2e-2 gate); PSUM stays f32.
x is cast to bf16 on the host; output DMAs back as bf16 and is cast to f32
on the host (halves HBM traffic both ways).

Sharding: data-parallel over batch, 2 images per core on 8 cores.
"""

import numpy as np

B, C, H, W = 16, 32, 64, 64
O, P, KH, KW = 128, 5, 3, 3
NCORES = 8
IPC = B // NCORES            # images per core
HP, WP = H + 2, W + 2        # padded image (pad=1)
RT = 8                       # output rows per L-tile
NT = H // RT                 # L-tiles per image
K2 = KH * KW
ATOL = 1e-5
RTOL = 1e-5
ROWTILE = False


# ---------------------------------------------------------------- host math


def _isclose_np(a, b):
    return np.abs(a - b) <= np.float32(ATOL) + np.float32(RTOL) * np.abs(b)


def _reference_np(x, weights, bias, positions):
    """Direct numpy port of the reference (fallback path)."""
    EPS = 1e-6
    Bn, Cn, Hn, Wn = x.shape
    On, _, Pn, KHn, KWn = weights.shape
    xp = np.pad(x, ((0, 0), (0, 0), (1, 1), (1, 1)))
    cols = [
        xp[:, :, i : i + Hn, j : j + Wn] for i in range(KHn) for j in range(KWn)
    ]
    pat = np.stack(cols, axis=2)
    v = pat.reshape(Bn, Cn, KHn * KWn, Hn * Wn).astype(np.float32)

    left, right = positions[:-1], positions[1:]
    denom = right - left
    denom = np.where(denom == 0, np.float32(EPS), denom)
    varc = (1.0 / denom).astype(np.float32)
    const = (-left * varc).astype(np.float32)

    m_first = _isclose_np(v, positions[0])
    m_last = _isclose_np(v, positions[-1])
    in_range = (~(m_first | m_last)) & (v >= positions[0]) & (v <= positions[-1])

    coeff = np.zeros(v.shape + (Pn,), np.float32)
    coeff[..., 0] += m_first.astype(np.float32)
    coeff[..., Pn - 1] += m_last.astype(np.float32)
    for p in range(Pn - 1):
        m = (in_range & (v >= positions[p]) & (v < positions[p + 1])).astype(
            np.float32
        )
        t = v * varc[p] + const[p]
        coeff[..., p] += m * (1.0 - t)
        coeff[..., p + 1] += m * t

    Wk = np.transpose(weights, (0, 1, 3, 4, 2)).reshape(On, Cn, KHn * KWn, Pn)
    ident = np.all(np.abs(Wk - 1.0) <= np.float32(ATOL + RTOL), axis=-1)
    Wk_eff = np.where(ident[..., None], np.float32(0.0), Wk)

    out = np.einsum("bcklp,ockp->bol", coeff, Wk_eff, optimize=True)
    out = out + np.einsum(
        "bckl,ock->bol", v, ident.astype(np.float32), optimize=True
    )
    out = out + bias[None, :, None]
    return out.reshape(Bn, On, Hn, Wn).astype(np.float32)


def _host_weights(weights, bias):
    """Fold c3 and c2 away.  Returns (wpair [128,3,O] bf16, wsing [128,2,O]
    bf16, bias_eff [O,1] f32, ident_any).

    lhsT row layout matches Y's partition groups [v, g, v-shift, g-shift]:
      wpair[:, kw, :] = [B(0,kw); A'(0,kw); B(1,kw); A'(1,kw)]
      wsing[:, 0, :]  = [B(2,0);  A'(2,0);  B(2,1);  A'(2,1)]
      wsing[:, 1, :]  = [B(2,2);  A'(2,2);  0;       0      ]
    """
    import ml_dtypes

    bf16 = ml_dtypes.bfloat16
    Wk = np.transpose(weights, (0, 1, 3, 4, 2)).reshape(O, C, K2, P)
    ident = np.all(np.abs(Wk - 1.0) <= np.float32(ATOL + RTOL), axis=-1)
    ident_any = bool(ident.any())
    Wk_eff = np.where(ident[..., None], np.float32(0.0), Wk).astype(np.float64)
    W2 = Wk_eff[:, :, :, 2]
    W3 = Wk_eff[:, :, :, 3]
    W4 = Wk_eff[:, :, :, 4]
    Ap = (2.0 * (W2 + W4 - 2.0 * W3)).astype(np.float32)   # [O,C,K2] g-weights
    Bw = (-2.0 * (W2 - W3)).astype(np.float32)             # [O,C,K2] v-weights
    # transpose to [C, K2, O] for lhsT rows
    ApT = np.ascontiguousarray(Ap.transpose(1, 2, 0))
    BwT = np.ascontiguousarray(Bw.transpose(1, 2, 0))

    def tap(kh, kw):
        return kh * KW + kw

    wpair = np.zeros((128, KW, O), np.float32)
    for kw in range(KW):
        wpair[0:32, kw] = BwT[:, tap(0, kw)]
        wpair[32:64, kw] = ApT[:, tap(0, kw)]
        wpair[64:96, kw] = BwT[:, tap(1, kw)]
        wpair[96:128, kw] = ApT[:, tap(1, kw)]
    wsing = np.zeros((128, 3, O), np.float32)
    wsing[0:32, 0] = BwT[:, tap(2, 0)]
    wsing[32:64, 0] = ApT[:, tap(2, 0)]
    wsing[64:96, 0] = BwT[:, tap(2, 1)]
    wsing[96:128, 0] = ApT[:, tap(2, 1)]
    wsing[0:32, 1] = BwT[:, tap(2, 2)]
    wsing[32:64, 1] = ApT[:, tap(2, 2)]
    wsing[0:32, 2] = BwT[:, tap(2, 1)]
    wsing[32:64, 2] = ApT[:, tap(2, 1)]

    bias_eff = (bias.astype(np.float64) + W2.sum(axis=(1, 2))).astype(np.float32)
    return (
        np.ascontiguousarray(wpair.astype(bf16)),
        np.ascontiguousarray(wsing.astype(bf16)),
        np.ascontiguousarray(bias_eff.reshape(O, 1)),
        ident_any,
    )


# ---------------------------------------------------------------- device IR


def _build_nc():
    import concourse.tile as tile
    from concourse import bacc, mybir

    f32 = mybir.dt.float32
    bf16 = mybir.dt.bfloat16
    Alu = mybir.AluOpType
    Act = mybir.ActivationFunctionType

    nc = bacc.Bacc("TRN2", target_bir_lowering=False, debug=False,
                   num_devices=NCORES)
    x_d = nc.dram_tensor("xb", [IPC, C, H, W], bf16, kind="ExternalInput").ap()
    wp_d = nc.dram_tensor("wpair", [128, KW, O], bf16, kind="ExternalInput").ap()
    ws_d = nc.dram_tensor("wsing", [128, 3, O], bf16, kind="ExternalInput").ap()
    b_d = nc.dram_tensor("bias", [O, 1], f32, kind="ExternalInput").ap()
    o_d = nc.dram_tensor("out", [IPC, O, H, W], bf16, kind="ExternalOutput").ap()

    with tile.TileContext(nc) as tc:
        with (
            tc.tile_pool(name="const", bufs=1) as constp,
            tc.tile_pool(name="scratch", bufs=1) as scrp,
            tc.tile_pool(name="ybuf", bufs=1) as ybufp,
            tc.tile_pool(name="psum", bufs=1, space="PSUM") as psump,
            tc.tile_pool(name="osb", bufs=4) as osbp,
        ):
            # ---- x loads first (critical path) ----
            XB = scrp.tile([C, IPC, H, W], bf16)     # x, both images
            for i in range(IPC):
                nc.sync.dma_start(XB[:, i], x_d[i])

            # pull the ACT table load off the critical path
            tiny = constp.tile([C, 1], f32)
            nc.gpsimd.memset(tiny[:], 0.0)
            nc.scalar.activation(tiny[:], tiny[:], Act.Relu, bias=0.0, scale=1.0)

            # PE warmup: dummy matmuls keep HAM busy until the real stream
            # starts (a cold PE runs at 1.2GHz for the first ~3.4us)
            zb = constp.tile([128, 512], bf16)
            nc.gpsimd.memset(zb[:], 0.0)
            warm_ctr = [0]

            def warm(nmm, rhs=None):
                """Dummy matmuls (results never read). rhs gates when the
                batch can start, chaining PE busy-ness across the phi phase."""
                w = warm_ctr[0]
                warm_ctr[0] += 1
                pw = psump.tile(
                    [O, 512], f32, name=f"ps_warm{w}", tag=f"ps{w % 2}"
                )
                r = zb[:] if rhs is None else rhs
                kp = r.shape[0]
                for j in range(nmm):
                    nc.tensor.matmul(
                        pw[:], zb[0:kp, 0:128], r,
                        start=(j % 8 == 0), stop=(j % 8 == 7 or j == nmm - 1),
                    )

            warm(6)

            # ---- weights + bias ----
            wp_sb = constp.tile([128, KW, O], bf16)
            nc.sync.dma_start(wp_sb[:], wp_d[:])
            ws_sb = constp.tile([128, 3, O], bf16)
            nc.sync.dma_start(ws_sb[:], ws_d[:])
            b_sb = constp.tile([O, 1], f32)
            nc.sync.dma_start(b_sb[:], b_d[:])

            # ---- g = relu(v - 0.5) for both images ----
            C4 = scrp.tile([C, IPC, H, W], bf16)
            neghalf = constp.tile([C, 1], f32)
            nc.gpsimd.memset(neghalf[:], -0.5)
            # split scalar/vector per image to halve latency
            for i in range(IPC):
                nc.scalar.activation(
                    C4[:, i, 0 : H // 2], XB[:, i, 0 : H // 2], Act.Relu,
                    bias=neghalf[:], scale=1.0,
                )
                nc.vector.tensor_scalar(
                    C4[:, i, H // 2 : H], XB[:, i, H // 2 : H],
                    0.5, 0.0, Alu.subtract, Alu.max,
                )
                if i == 0:
                    warm(5, XB[:, 0, 0:RT])

            # ---- Y buffers: [v, g, v-up1, g-up1] x 128 partitions ----
            # all border values are 0 for both planes
            Y = [
                ybufp.tile([128, HP, WP], bf16, name=f"Y{i}")
                for i in range(IPC)
            ]
            for y in Y:
                # rows 0 & HP-1 for unshifted groups, row H for shifted
                nc.gpsimd.memset(y[0:64, 0, :], 0.0)
                nc.gpsimd.memset(y[0:64, HP - 1, :], 0.0)
                nc.gpsimd.memset(y[64:128, H, :], 0.0)
                # col borders, all 128 partitions
                nc.gpsimd.memset(y[:, 0 : HP - 1, 0], 0.0)
                nc.gpsimd.memset(y[:, 0 : HP - 1, WP - 1], 0.0)

            def gather(i):
                y = Y[i]
                nc.sync.dma_start(y[0:32, 1 : H + 1, 1 : W + 1], XB[:, i])
                nc.sync.dma_start(y[32:64, 1 : H + 1, 1 : W + 1], C4[:, i])
                nc.sync.dma_start(y[64:96, 0:H, 1 : W + 1], XB[:, i])
                nc.sync.dma_start(y[96:128, 0:H, 1 : W + 1], C4[:, i])

            gather(0)
            warm(5, C4[:, 0, 0:RT])
            gather(1)

            # ---- GEMM: 5 matmul slots per tile, taps outer ----
            for i in range(IPC):
                y = Y[i]
                pss = [
                    psump.tile([O, RT * W], f32, name=f"ps_i{i}t{t}",
                               tag=f"ps{t}")
                    for t in range(NT)
                ]
                # slots 1-3: paired taps (0,kw)+(1,kw), K=128
                for kw in range(KW):
                    for t in range(NT):
                        nc.tensor.matmul(
                            pss[t][:], wp_sb[:, kw, :],
                            y[:, t * RT : t * RT + RT, kw : kw + W],
                            start=(kw == 0), stop=False,
                        )
                # slot 4: taps (2,0) & (2,1) as concurrent row-tiled K=64
                for t in range(NT):
                    nc.tensor.matmul(
                        pss[t][:], ws_sb[0:64, 0, :],
                        y[0:64, t * RT + 2 : t * RT + 2 + RT, 0:W],
                        start=False, stop=False,
                    )
                    if ROWTILE:
                        nc.tensor.matmul(
                            pss[t][:], ws_sb[64:128, 0, :],
                            y[64:128, t * RT + 1 : t * RT + 1 + RT, 1 : 1 + W],
                            start=False, stop=False,
                        )
                    else:
                        nc.tensor.matmul(
                            pss[t][:], ws_sb[0:64, 2, :],
                            y[0:64, t * RT + 2 : t * RT + 2 + RT, 1 : 1 + W],
                            start=False, stop=False,
                        )
                # slot 5: tap (2,2), K=64
                for t in range(NT):
                    nc.tensor.matmul(
                        pss[t][:], ws_sb[0:64, 1, :],
                        y[0:64, t * RT + 2 : t * RT + 2 + RT, 2 : 2 + W],
                        start=False, stop=True,
                    )
                # drain: bias add, bf16 out, DMA per tile
                for t in range(NT):
                    osb = osbp.tile([O, RT * W], bf16, name="osb")
                    if t % 2 == 0:
                        nc.scalar.activation(
                            osb[:], pss[t][:], Act.Identity,
                            bias=b_sb[:, 0:1], scale=1.0,
                        )
                    else:
                        nc.vector.tensor_scalar(
                            osb[:], pss[t][:], b_sb[:, 0:1], None, Alu.add
                        )
                    nc.sync.dma_start(
                        o_d[i, :, t * RT : (t + 1) * RT, :],
                        osb[:].rearrange("o (r w) -> o r w", r=RT),
                    )
    nc.compile()
    return nc


# ---------------------------------------------------------------- entry


def _prep(inputs):
    x = np.ascontiguousarray(np.asarray(inputs["x"], dtype=np.float32))
    weights = np.ascontiguousarray(np.asarray(inputs["weights"], dtype=np.float32))
    bias = np.ascontiguousarray(np.asarray(inputs["bias"], dtype=np.float32))
    positions = np.ascontiguousarray(
        np.asarray(inputs["positions"], dtype=np.float32)
    )
    return x, weights, bias, positions


def _fast_path_ok(x, positions):
    expect = np.linspace(-1.0, 1.0, P, dtype=np.float32)
    return (
        x.shape == (B, C, H, W)
        and positions.shape == (P,)
        and np.array_equal(positions, expect)
        and float(x.min()) >= 0.0
        and float(x.max()) <= 1.0
    )


def kernel(**inputs):
    import ml_dtypes

    x, weights, bias, positions = _prep(inputs)
    if not _fast_path_ok(x, positions):
        return _reference_np(x, weights, bias, positions)

    wpair, wsing, bias_eff, ident_any = _host_weights(weights, bias)
    if ident_any:
        # identity-shortcut weights present: needs the raw-v plane; use the
        # exact fallback rather than a rarely-exercised device path
        return _reference_np(x, weights, bias, positions)

    from concourse.bass_utils import run_bass_kernel_spmd

    nc = _build_nc()
    xb = np.ascontiguousarray(x.astype(ml_dtypes.bfloat16))
    in_maps = [
        {"xb": xb[i * IPC : (i + 1) * IPC],
         "wpair": wpair, "wsing": wsing, "bias": bias_eff}
        for i in range(NCORES)
    ]
    res = run_bass_kernel_spmd(nc, in_maps, core_ids=list(range(NCORES)))
    out = np.concatenate([res.results[i]["out"] for i in range(NCORES)], axis=0)
    return np.ascontiguousarray(out.astype(np.float32))


# ------------------------------------------------------------ dev utilities


def _run_sim(inputs):
    """CoreSim single-core run (images 0..IPC-1) for correctness debugging."""
    import ml_dtypes
    from concourse.bass_interp import CoreSim

    x, weights, bias, positions = _prep(inputs)
    assert _fast_path_ok(x, positions)
    wpair, wsing, bias_eff, ident_any = _host_weights(weights, bias)
    assert not ident_any
    nc = _build_nc()
    sim = CoreSim(nc)
    sim.tensor("xb")[:] = x[:IPC].astype(ml_dtypes.bfloat16)
    sim.tensor("wpair")[:] = wpair
    sim.tensor("wsing")[:] = wsing
    sim.tensor("bias")[:] = bias_eff
    sim.simulate()
    return np.array(sim.tensor("out")).astype(np.float32)


# revision 7
# speedup vs baseline: 2.2183x; 1.4691x over previous
"""Trainium2 Bass kernel for CustomPositionsPiecewiseConv2d.

Math: for knots [-1,-.5,0,.5,1] and x in [0,1] the active coefficients are
c2 = relu(1-2v), c4 = relu(2v-1), c3 = 1-c2-c4.  Folding c3 away and then
eliminating c2 via the identity c2 - c4 = 1 - 2v gives a TWO-plane GEMM:

    out = sum_ck  g(v) * A'[c,k,o]  +  v * B[c,k,o]  + bias_eff[o]
    g(v) = relu(v - 0.5)            (= c4/2)
    A'   = 2*(W2 + W4 - 2*W3)
    B    = -2*(W2 - W3)
    bias_eff = bias + sum_ck W2

The v-plane is just (padded, bf16-cast) x — no compute — and g needs one
activation op.  Both planes are 0 on the zero-padding border.

Tap packing: Y holds [v, g, v-shift-up-1row, g-shift-up-1row] across 128
partitions, so one K=128 matmul covers taps (0,kw) and (1,kw) at once.
Taps (2,0)/(2,1) run as two concurrent K=64 row-tiled matmuls (array rows
0-63 / 64-127); tap (2,2) is a lone K=64 matmul.  5 matmul slots per
output tile instead of 9 (theoretical floor 4.5).

Everything is bf16 on the PE (err ~3e-3 << the 2e-2 gate); PSUM stays f32.
x is cast to bf16 on the host; output DMAs back as bf16 and is cast to f32
on the host (halves HBM traffic both ways).

Sharding: data-parallel over batch, 2 images per core on 8 cores.
"""

import numpy as np

B, C, H, W = 16, 32, 64, 64
O, P, KH, KW = 128, 5, 3, 3
NCORES = 8
IPC = B // NCORES            # images per core
HP, WP = H + 2, W + 2        # padded image (pad=1)
RT = 8                       # output rows per L-tile
NT = H // RT                 # L-tiles per image
K2 = KH * KW
ATOL = 1e-5
RTOL = 1e-5
ROWTILE = False


# ---------------------------------------------------------------- host math


def _isclose_np(a, b):
    return np.abs(a - b) <= np.float32(ATOL) + np.float32(RTOL) * np.abs(b)


def _reference_np(x, weights, bias, positions):
    """Direct numpy port of the reference (fallback path)."""
    EPS = 1e-6
    Bn, Cn, Hn, Wn = x.shape
    On, _, Pn, KHn, KWn = weights.shape
    xp = np.pad(x, ((0, 0), (0, 0), (1, 1), (1, 1)))
    cols = [
        xp[:, :, i : i + Hn, j : j + Wn] for i in range(KHn) for j in range(KWn)
    ]
    pat = np.stack(cols, axis=2)
    v = pat.reshape(Bn, Cn, KHn * KWn, Hn * Wn).astype(np.float32)

    left, right = positions[:-1], positions[1:]
    denom = right - left
    denom = np.where(denom == 0, np.float32(EPS), denom)
    varc = (1.0 / denom).astype(np.float32)
    const = (-left * varc).astype(np.float32)

    m_first = _isclose_np(v, positions[0])
    m_last = _isclose_np(v, positions[-1])
    in_range = (~(m_first | m_last)) & (v >= positions[0]) & (v <= positions[-1])

    coeff = np.zeros(v.shape + (Pn,), np.float32)
    coeff[..., 0] += m_first.astype(np.float32)
    coeff[..., Pn - 1] += m_last.astype(np.float32)
    for p in range(Pn - 1):
        m = (in_range & (v >= positions[p]) & (v < positions[p + 1])).astype(
            np.float32
        )
        t = v * varc[p] + const[p]
        coeff[..., p] += m * (1.0 - t)
        coeff[..., p + 1] += m * t

    Wk = np.transpose(weights, (0, 1, 3, 4, 2)).reshape(On, Cn, KHn * KWn, Pn)
    ident = np.all(np.abs(Wk - 1.0) <= np.float32(ATOL + RTOL), axis=-1)
    Wk_eff = np.where(ident[..., None], np.float32(0.0), Wk)

    out = np.einsum("bcklp,ockp->bol", coeff, Wk_eff, optimize=True)
    out = out + np.einsum(
        "bckl,ock->bol", v, ident.astype(np.float32), optimize=True
    )
    out = out + bias[None, :, None]
    return out.reshape(Bn, On, Hn, Wn).astype(np.float32)


def _host_weights(weights, bias):
    """Fold c3 and c2 away.  Returns (wpair [128,3,O] bf16, wsing [128,2,O]
    bf16, bias_eff [O,1] f32, ident_any).

    lhsT row layout matches Y's partition groups [v, g, v-shift, g-shift]:
      wpair[:, kw, :] = [B(0,kw); A'(0,kw); B(1,kw); A'(1,kw)]
      wsing[:, 0, :]  = [B(2,0);  A'(2,0);  B(2,1);  A'(2,1)]
      wsing[:, 1, :]  = [B(2,2);  A'(2,2);  0;       0      ]
    """
    import ml_dtypes

    bf16 = ml_dtypes.bfloat16
    Wk = np.transpose(weights, (0, 1, 3, 4, 2)).reshape(O, C, K2, P)
    ident = np.all(np.abs(Wk - 1.0) <= np.float32(ATOL + RTOL), axis=-1)
    ident_any = bool(ident.any())
    Wk_eff = np.where(ident[..., None], np.float32(0.0), Wk).astype(np.float64)
    W2 = Wk_eff[:, :, :, 2]
    W3 = Wk_eff[:, :, :, 3]
    W4 = Wk_eff[:, :, :, 4]
    Ap = (2.0 * (W2 + W4 - 2.0 * W3)).astype(np.float32)   # [O,C,K2] g-weights
    Bw = (-2.0 * (W2 - W3)).astype(np.float32)             # [O,C,K2] v-weights
    # transpose to [C, K2, O] for lhsT rows
    ApT = np.ascontiguousarray(Ap.transpose(1, 2, 0))
    BwT = np.ascontiguousarray(Bw.transpose(1, 2, 0))

    def tap(kh, kw):
        return kh * KW + kw

    wpair = np.zeros((128, KW, O), np.float32)
    for kw in range(KW):
        wpair[0:32, kw] = BwT[:, tap(0, kw)]
        wpair[32:64, kw] = ApT[:, tap(0, kw)]
        wpair[64:96, kw] = BwT[:, tap(1, kw)]
        wpair[96:128, kw] = ApT[:, tap(1, kw)]
    wsing = np.zeros((128, 3, O), np.float32)
    wsing[0:32, 0] = BwT[:, tap(2, 0)]
    wsing[32:64, 0] = ApT[:, tap(2, 0)]
    wsing[64:96, 0] = BwT[:, tap(2, 1)]
    wsing[96:128, 0] = ApT[:, tap(2, 1)]
    wsing[0:32, 1] = BwT[:, tap(2, 2)]
    wsing[32:64, 1] = ApT[:, tap(2, 2)]
    wsing[0:32, 2] = BwT[:, tap(2, 1)]
    wsing[32:64, 2] = ApT[:, tap(2, 1)]

    bias_eff = (bias.astype(np.float64) + W2.sum(axis=(1, 2))).astype(np.float32)
    return (
        np.ascontiguousarray(wpair.astype(bf16)),
        np.ascontiguousarray(wsing.astype(bf16)),
        np.ascontiguousarray(bias_eff.reshape(O, 1)),
        ident_any,
    )


# ---------------------------------------------------------------- device IR


def _build_nc():
    import concourse.tile as tile
    from concourse import bacc, mybir

    f32 = mybir.dt.float32
    bf16 = mybir.dt.bfloat16
    Alu = mybir.AluOpType
    Act = mybir.ActivationFunctionType

    WG = W + 1                   # row pitch: 64 data + 1 zero gap
    NIMG = H * WG                # flat elems per image plane (no pad rows)
    # Y flat layout: elem 0 = leading zero pad; logical row r at
    # [1 + r*WG, 1 + r*WG + W), gap at +W.  Unshifted groups: row 0 = top
    # pad, rows 1..H = image, row H+1 = bottom pad.  Shifted groups: row r
    # = image row r (i.e. shifted up by one), row H = bottom pad.
    YSZ = 1 + (HP + 1) * WG      # extra slack row so AP slices stay in bounds

    nc = bacc.Bacc("TRN2", target_bir_lowering=False, debug=False,
                   num_devices=NCORES)
    x_d = nc.dram_tensor("xg", [IPC, C, H, WG], bf16, kind="ExternalInput").ap()
    wp_d = nc.dram_tensor("wpair", [128, KW, O], bf16, kind="ExternalInput").ap()
    ws_d = nc.dram_tensor("wsing", [128, 3, O], bf16, kind="ExternalInput").ap()
    b_d = nc.dram_tensor("bias", [O, 1], f32, kind="ExternalInput").ap()
    o_d = nc.dram_tensor("out", [IPC, O, H, W], bf16, kind="ExternalOutput").ap()

    with tile.TileContext(nc) as tc:
        with (
            tc.tile_pool(name="const", bufs=1) as constp,
            tc.tile_pool(name="scratch", bufs=1) as scrp,
            tc.tile_pool(name="ybuf", bufs=1) as ybufp,
            tc.tile_pool(name="psum", bufs=1, space="PSUM") as psump,
            tc.tile_pool(name="osb", bufs=4) as osbp,
        ):
            Y = [
                ybufp.tile([128, YSZ], bf16, name=f"Y{i}")
                for i in range(IPC)
            ]
            C4 = scrp.tile([C, IPC, NIMG], bf16)

            # ---- x-plane loads first (critical path, SP queue) ----
            # unshifted x-plane: rows 1..H
            nc.sync.dma_start(Y[0][0:32, 1 + WG : 1 + WG + NIMG], x_d[0])
            wp_sb = constp.tile([128, KW, O], bf16)
            nc.sync.dma_start(wp_sb[:], wp_d[:])
            # shifted x-plane: rows 0..H-1
            nc.sync.dma_start(Y[0][64:96, 1 : 1 + NIMG], x_d[0])
            ws_sb = constp.tile([128, 3, O], bf16)
            nc.sync.dma_start(ws_sb[:], ws_d[:])
            b_sb = constp.tile([O, 1], f32)
            nc.sync.dma_start(b_sb[:], b_d[:])
            nc.sync.dma_start(Y[1][0:32, 1 + WG : 1 + WG + NIMG], x_d[1])
            nc.sync.dma_start(Y[1][64:96, 1 : 1 + NIMG], x_d[1])

            # pull the ACT table load off the critical path
            tiny = constp.tile([C, 1], f32)
            nc.gpsimd.memset(tiny[:], 0.0)
            nc.scalar.activation(tiny[:], tiny[:], Act.Relu, bias=0.0, scale=1.0)
            neghalf = constp.tile([C, 1], f32)
            nc.gpsimd.memset(neghalf[:], -0.5)

            # Y zero pads (all tiny contiguous strips)
            for y in Y:
                nc.gpsimd.memset(y[:, 0:1], 0.0)                  # leading pad
                nc.gpsimd.memset(y[0:64, 1 : 1 + WG], 0.0)        # top pad row
                bot = 1 + (H + 1) * WG
                nc.gpsimd.memset(y[0:64, bot : bot + WG], 0.0)    # bottom pad
                sbot = 1 + H * WG
                nc.gpsimd.memset(y[64:128, sbot : sbot + WG], 0.0)

            # PE warmup: dummy matmuls keep HAM busy until the real stream
            # starts (a cold PE runs at 1.2GHz for the first ~3.4us)
            zb = constp.tile([128, 512], bf16)
            nc.gpsimd.memset(zb[:], 0.0)
            warm_ctr = [0]

            def warm(nmm, rhs=None):
                w = warm_ctr[0]
                warm_ctr[0] += 1
                pw = psump.tile(
                    [O, 512], f32, name=f"ps_warm{w}", tag=f"ps{w % 2}"
                )
                r = zb[:] if rhs is None else rhs
                kp = r.shape[0]
                for j in range(nmm):
                    nc.tensor.matmul(
                        pw[:], zb[0:kp, 0:128], r,
                        start=(j % 8 == 0), stop=(j % 8 == 7 or j == nmm - 1),
                    )

            warm(5)

            # ---- g = relu(v - 0.5), computed from the in-Y x-plane ----
            # split scalar (24 rows) / vector (40 rows) to cut latency;
            # gap columns map 0 -> 0 so C4 inherits correct gaps
            SR = 24 * WG
            for i in range(IPC):
                src = Y[i][0:32, 1 + WG : 1 + WG + NIMG]
                nc.scalar.activation(
                    C4[:, i, 0:SR], src[:, 0:SR], Act.Relu,
                    bias=neghalf[:], scale=1.0,
                )
                nc.vector.tensor_scalar(
                    C4[:, i, SR:NIMG], src[:, SR:NIMG],
                    0.5, 0.0, Alu.subtract, Alu.max,
                )
                if i == 0:
                    warm(3, Y[0][0:32, 1 + WG : 1 + WG + 512])

            # c4 gathers on the scalar HWDGE queue (SP stays free for x/out)
            for i in range(IPC):
                nc.scalar.dma_start(
                    Y[i][32:64, 1 + WG : 1 + WG + NIMG], C4[:, i]
                )
                nc.scalar.dma_start(
                    Y[i][96:128, 1 : 1 + NIMG], C4[:, i]
                )
                if i == 0:
                    warm(2, C4[:, 0, 0:512])

            def ywin(y, p0, p1, r0, kw):
                """[p1-p0, RT, W] window: rows r0..r0+RT, cols kw-1..kw-1+W."""
                off = 1 + r0 * WG + (kw - 1)
                return y[p0:p1, off : off + RT * WG].rearrange(
                    "p (r c) -> p r c", r=RT
                )[:, :, 0:W]

            # ---- GEMM: taps outer, 8 PSUM banks = 8 row-tiles ----
            for i in range(IPC):
                y = Y[i]
                pss = [
                    psump.tile([O, RT * W], f32, name=f"ps_i{i}t{t}",
                               tag=f"ps{t}")
                    for t in range(NT)
                ]
                # slots 1-3: paired taps (0,kw)+(1,kw), K=128
                for kw in range(KW):
                    for t in range(NT):
                        nc.tensor.matmul(
                            pss[t][:], wp_sb[:, kw, :],
                            ywin(y, 0, 128, t * RT, kw),
                            start=(kw == 0), stop=False,
                        )
                # slot 4: tap (2,0) K=64 (+ (2,1) row-tiled if ROWTILE)
                for t in range(NT):
                    nc.tensor.matmul(
                        pss[t][:], ws_sb[0:64, 0, :],
                        ywin(y, 0, 64, t * RT + 2, 0),
                        start=False, stop=False,
                    )
                    if ROWTILE:
                        nc.tensor.matmul(
                            pss[t][:], ws_sb[64:128, 0, :],
                            ywin(y, 64, 128, t * RT + 1, 1),
                            start=False, stop=False,
                        )
                    else:
                        nc.tensor.matmul(
                            pss[t][:], ws_sb[0:64, 2, :],
                            ywin(y, 0, 64, t * RT + 2, 1),
                            start=False, stop=False,
                        )
                # slot 5: tap (2,2), K=64
                for t in range(NT):
                    nc.tensor.matmul(
                        pss[t][:], ws_sb[0:64, 1, :],
                        ywin(y, 0, 64, t * RT + 2, 2),
                        start=False, stop=True,
                    )
                # drain: bias add, bf16 out, DMA per tile (SP queue)
                for t in range(NT):
                    osb = osbp.tile([O, RT * W], bf16, name="osb")
                    if t % 2 == 0:
                        nc.scalar.activation(
                            osb[:], pss[t][:], Act.Identity,
                            bias=b_sb[:, 0:1], scale=1.0,
                        )
                    else:
                        nc.vector.tensor_scalar(
                            osb[:], pss[t][:], b_sb[:, 0:1], None, Alu.add
                        )
                    nc.sync.dma_start(
                        o_d[i, :, t * RT : (t + 1) * RT, :],
                        osb[:].rearrange("o (r w) -> o r w", r=RT),
                    )
    nc.compile()
    return nc


# ---------------------------------------------------------------- entry


def _prep(inputs):
    x = np.ascontiguousarray(np.asarray(inputs["x"], dtype=np.float32))
    weights = np.ascontiguousarray(np.asarray(inputs["weights"], dtype=np.float32))
    bias = np.ascontiguousarray(np.asarray(inputs["bias"], dtype=np.float32))
    positions = np.ascontiguousarray(
        np.asarray(inputs["positions"], dtype=np.float32)
    )
    return x, weights, bias, positions


def _fast_path_ok(x, positions):
    expect = np.linspace(-1.0, 1.0, P, dtype=np.float32)
    return (
        x.shape == (B, C, H, W)
        and positions.shape == (P,)
        and np.array_equal(positions, expect)
        and float(x.min()) >= 0.0
        and float(x.max()) <= 1.0
    )


def kernel(**inputs):
    import ml_dtypes

    x, weights, bias, positions = _prep(inputs)
    if not _fast_path_ok(x, positions):
        return _reference_np(x, weights, bias, positions)

    wpair, wsing, bias_eff, ident_any = _host_weights(weights, bias)
    if ident_any:
        # identity-shortcut weights present: needs the raw-v plane; use the
        # exact fallback rather than a rarely-exercised device path
        return _reference_np(x, weights, bias, positions)

    from concourse.bass_utils import run_bass_kernel_spmd

    nc = _build_nc()
    xg = np.zeros((B, C, H, W + 1), dtype=ml_dtypes.bfloat16)
    xg[:, :, :, :W] = x.astype(ml_dtypes.bfloat16)
    xg = np.ascontiguousarray(xg)
    in_maps = [
        {"xg": xg[i * IPC : (i + 1) * IPC],
         "wpair": wpair, "wsing": wsing, "bias": bias_eff}
        for i in range(NCORES)
    ]
    res = run_bass_kernel_spmd(nc, in_maps, core_ids=list(range(NCORES)))
    out = np.concatenate([res.results[i]["out"] for i in range(NCORES)], axis=0)
    return np.ascontiguousarray(out.astype(np.float32))


# ------------------------------------------------------------ dev utilities


def _run_sim(inputs):
    """CoreSim single-core run (images 0..IPC-1) for correctness debugging."""
    import ml_dtypes
    from concourse.bass_interp import CoreSim

    x, weights, bias, positions = _prep(inputs)
    assert _fast_path_ok(x, positions)
    wpair, wsing, bias_eff, ident_any = _host_weights(weights, bias)
    assert not ident_any
    nc = _build_nc()
    sim = CoreSim(nc)
    xg = np.zeros((IPC, C, H, W + 1), dtype=ml_dtypes.bfloat16)
    xg[:, :, :, :W] = x[:IPC].astype(ml_dtypes.bfloat16)
    sim.tensor("xg")[:] = xg
    sim.tensor("wpair")[:] = wpair
    sim.tensor("wsing")[:] = wsing
    sim.tensor("bias")[:] = bias_eff
    sim.simulate()
    return np.array(sim.tensor("out")).astype(np.float32)


# revision 10
# speedup vs baseline: 2.4574x; 1.1078x over previous
"""Trainium2 Bass kernel for CustomPositionsPiecewiseConv2d.

Math: for knots [-1,-.5,0,.5,1] and x in [0,1] the active coefficients are
c2 = relu(1-2v), c4 = relu(2v-1), c3 = 1-c2-c4.  Folding c3 away and then
eliminating c2 via the identity c2 - c4 = 1 - 2v gives a TWO-plane GEMM:

    out = sum_ck  g(v) * A'[c,k,o]  +  v * B[c,k,o]  + bias_eff[o]
    g(v) = relu(v - 0.5)            (= c4/2)
    A'   = 2*(W2 + W4 - 2*W3)
    B    = -2*(W2 - W3)
    bias_eff = bias + sum_ck W2

The v-plane is just (padded, bf16-cast) x — no compute — and g needs one
activation op.  Both planes are 0 on the zero-padding border.

Tap packing: Y holds [v, g, v-shift-up-1row, g-shift-up-1row] across 128
partitions, so one K=128 matmul covers taps (0,kw) and (1,kw) at once.
Taps (2,0)/(2,1) run as two concurrent K=64 row-tiled matmuls (array rows
0-63 / 64-127); tap (2,2) is a lone K=64 matmul.  5 matmul slots per
output tile instead of 9 (theoretical floor 4.5).

Everything is bf16 on the PE (err ~3e-3 << the 2e-2 gate); PSUM stays f32.
x is cast to bf16 on the host; output DMAs back as bf16 and is cast to f32
on the host (halves HBM traffic both ways).

Sharding: data-parallel over batch, 2 images per core on 8 cores.
"""

import numpy as np

B, C, H, W = 16, 32, 64, 64
O, P, KH, KW = 128, 5, 3, 3
NCORES = 8
IPC = B // NCORES            # images per core
HP, WP = H + 2, W + 2        # padded image (pad=1)
RT = 8                       # output rows per L-tile
NT = H // RT                 # L-tiles per image
K2 = KH * KW
ATOL = 1e-5
RTOL = 1e-5
ROWTILE = False


# ---------------------------------------------------------------- host math


def _isclose_np(a, b):
    return np.abs(a - b) <= np.float32(ATOL) + np.float32(RTOL) * np.abs(b)


def _reference_np(x, weights, bias, positions):
    """Direct numpy port of the reference (fallback path)."""
    EPS = 1e-6
    Bn, Cn, Hn, Wn = x.shape
    On, _, Pn, KHn, KWn = weights.shape
    xp = np.pad(x, ((0, 0), (0, 0), (1, 1), (1, 1)))
    cols = [
        xp[:, :, i : i + Hn, j : j + Wn] for i in range(KHn) for j in range(KWn)
    ]
    pat = np.stack(cols, axis=2)
    v = pat.reshape(Bn, Cn, KHn * KWn, Hn * Wn).astype(np.float32)

    left, right = positions[:-1], positions[1:]
    denom = right - left
    denom = np.where(denom == 0, np.float32(EPS), denom)
    varc = (1.0 / denom).astype(np.float32)
    const = (-left * varc).astype(np.float32)

    m_first = _isclose_np(v, positions[0])
    m_last = _isclose_np(v, positions[-1])
    in_range = (~(m_first | m_last)) & (v >= positions[0]) & (v <= positions[-1])

    coeff = np.zeros(v.shape + (Pn,), np.float32)
    coeff[..., 0] += m_first.astype(np.float32)
    coeff[..., Pn - 1] += m_last.astype(np.float32)
    for p in range(Pn - 1):
        m = (in_range & (v >= positions[p]) & (v < positions[p + 1])).astype(
            np.float32
        )
        t = v * varc[p] + const[p]
        coeff[..., p] += m * (1.0 - t)
        coeff[..., p + 1] += m * t

    Wk = np.transpose(weights, (0, 1, 3, 4, 2)).reshape(On, Cn, KHn * KWn, Pn)
    ident = np.all(np.abs(Wk - 1.0) <= np.float32(ATOL + RTOL), axis=-1)
    Wk_eff = np.where(ident[..., None], np.float32(0.0), Wk)

    out = np.einsum("bcklp,ockp->bol", coeff, Wk_eff, optimize=True)
    out = out + np.einsum(
        "bckl,ock->bol", v, ident.astype(np.float32), optimize=True
    )
    out = out + bias[None, :, None]
    return out.reshape(Bn, On, Hn, Wn).astype(np.float32)


def _host_weights(weights, bias):
    """Fold c3 and c2 away.  Returns (wpair [128,3,O] bf16, wsing [128,2,O]
    bf16, bias_eff [O,1] f32, ident_any).

    lhsT row layout matches Y's partition groups [v, g, v-shift, g-shift]:
      wpair[:, kw, :] = [B(0,kw); A'(0,kw); B(1,kw); A'(1,kw)]
      wsing[:, 0, :]  = [B(2,0);  A'(2,0);  B(2,1);  A'(2,1)]
      wsing[:, 1, :]  = [B(2,2);  A'(2,2);  0;       0      ]
    """
    import ml_dtypes

    bf16 = ml_dtypes.bfloat16
    Wk = np.transpose(weights, (0, 1, 3, 4, 2)).reshape(O, C, K2, P)
    ident = np.all(np.abs(Wk - 1.0) <= np.float32(ATOL + RTOL), axis=-1)
    ident_any = bool(ident.any())
    Wk_eff = np.where(ident[..., None], np.float32(0.0), Wk).astype(np.float64)
    W2 = Wk_eff[:, :, :, 2]
    W3 = Wk_eff[:, :, :, 3]
    W4 = Wk_eff[:, :, :, 4]
    Ap = (2.0 * (W2 + W4 - 2.0 * W3)).astype(np.float32)   # [O,C,K2] g-weights
    Bw = (-2.0 * (W2 - W3)).astype(np.float32)             # [O,C,K2] v-weights
    # transpose to [C, K2, O] for lhsT rows
    ApT = np.ascontiguousarray(Ap.transpose(1, 2, 0))
    BwT = np.ascontiguousarray(Bw.transpose(1, 2, 0))

    def tap(kh, kw):
        return kh * KW + kw

    wpair = np.zeros((128, KW, O), np.float32)
    for kw in range(KW):
        wpair[0:32, kw] = BwT[:, tap(0, kw)]
        wpair[32:64, kw] = ApT[:, tap(0, kw)]
        wpair[64:96, kw] = BwT[:, tap(1, kw)]
        wpair[96:128, kw] = ApT[:, tap(1, kw)]
    wsing = np.zeros((128, 3, O), np.float32)
    wsing[0:32, 0] = BwT[:, tap(2, 0)]
    wsing[32:64, 0] = ApT[:, tap(2, 0)]
    wsing[64:96, 0] = BwT[:, tap(2, 1)]
    wsing[96:128, 0] = ApT[:, tap(2, 1)]
    wsing[0:32, 1] = BwT[:, tap(2, 2)]
    wsing[32:64, 1] = ApT[:, tap(2, 2)]
    wsing[0:32, 2] = BwT[:, tap(2, 1)]
    wsing[32:64, 2] = ApT[:, tap(2, 1)]

    bias_eff = (bias.astype(np.float64) + W2.sum(axis=(1, 2))).astype(np.float32)
    return (
        np.ascontiguousarray(wpair.astype(bf16)),
        np.ascontiguousarray(wsing.astype(bf16)),
        np.ascontiguousarray(bias_eff.reshape(O, 1)),
        ident_any,
    )


# ---------------------------------------------------------------- device IR


def _build_nc():
    import concourse.tile as tile
    from concourse import bacc, mybir

    f32 = mybir.dt.float32
    bf16 = mybir.dt.bfloat16
    Alu = mybir.AluOpType
    Act = mybir.ActivationFunctionType

    WG = W + 1                   # row pitch: 64 data + 1 zero gap
    NIMG = H * WG                # flat elems per image plane (no pad rows)
    # Y flat layout: elem 0 = leading zero pad; logical row r at
    # [1 + r*WG, 1 + r*WG + W), gap at +W.  Unshifted groups: row 0 = top
    # pad, rows 1..H = image, row H+1 = bottom pad.  Shifted groups: row r
    # = image row r (i.e. shifted up by one), row H = bottom pad.
    YSZ = 1 + (HP + 1) * WG      # extra slack row so AP slices stay in bounds

    nc = bacc.Bacc("TRN2", target_bir_lowering=False, debug=False,
                   num_devices=NCORES)
    x_d = nc.dram_tensor("xg", [IPC, C, H, WG], bf16, kind="ExternalInput").ap()
    wp_d = nc.dram_tensor("wpair", [128, KW, O], bf16, kind="ExternalInput").ap()
    ws_d = nc.dram_tensor("wsing", [128, 3, O], bf16, kind="ExternalInput").ap()
    b_d = nc.dram_tensor("bias", [O, 1], f32, kind="ExternalInput").ap()
    o_d = nc.dram_tensor("out", [IPC, O, H, W], bf16, kind="ExternalOutput").ap()

    with tile.TileContext(nc) as tc:
        with (
            tc.tile_pool(name="const", bufs=1) as constp,
            tc.tile_pool(name="ybuf", bufs=1) as ybufp,
            tc.tile_pool(name="psum", bufs=1, space="PSUM") as psump,
            tc.tile_pool(name="osb", bufs=4) as osbp,
        ):
            Y = [
                ybufp.tile([128, YSZ], bf16, name=f"Y{i}")
                for i in range(IPC)
            ]

            # ---- x loads (SP queue): all 4 partition groups from HBM ----
            # c4-input groups first (they gate the in-place relu)
            nc.sync.dma_start(Y[0][32:64, 1 + WG : 1 + WG + NIMG], x_d[0])
            nc.sync.dma_start(Y[0][96:128, 1 : 1 + NIMG], x_d[0])
            wp_sb = constp.tile([128, KW, O], bf16)
            nc.sync.dma_start(wp_sb[:], wp_d[:])
            nc.sync.dma_start(Y[0][0:32, 1 + WG : 1 + WG + NIMG], x_d[0])
            nc.sync.dma_start(Y[0][64:96, 1 : 1 + NIMG], x_d[0])
            ws_sb = constp.tile([128, 3, O], bf16)
            nc.sync.dma_start(ws_sb[:], ws_d[:])
            b_sb = constp.tile([O, 1], f32)
            nc.sync.dma_start(b_sb[:], b_d[:])
            nc.sync.dma_start(Y[1][32:64, 1 + WG : 1 + WG + NIMG], x_d[1])
            nc.sync.dma_start(Y[1][96:128, 1 : 1 + NIMG], x_d[1])
            nc.sync.dma_start(Y[1][0:32, 1 + WG : 1 + WG + NIMG], x_d[1])
            nc.sync.dma_start(Y[1][64:96, 1 : 1 + NIMG], x_d[1])

            # pull the ACT table load off the critical path
            tiny = constp.tile([C, 1], f32)
            nc.gpsimd.memset(tiny[:], 0.0)
            nc.scalar.activation(tiny[:], tiny[:], Act.Relu, bias=0.0, scale=1.0)
            neghalf = constp.tile([128, 1], f32)
            nc.gpsimd.memset(neghalf[:], -0.5)

            # Y zero pads (all tiny contiguous strips)
            for y in Y:
                nc.gpsimd.memset(y[:, 0:1], 0.0)                  # leading pad
                nc.gpsimd.memset(y[0:64, 1 : 1 + WG], 0.0)        # top pad row
                bot = 1 + (H + 1) * WG
                nc.gpsimd.memset(y[0:64, bot : bot + WG], 0.0)    # bottom pad
                sbot = 1 + H * WG
                nc.gpsimd.memset(y[64:128, sbot : sbot + WG], 0.0)

            # PE warmup: dummy matmuls keep HAM busy until the real stream
            # starts (a cold PE runs at 1.2GHz for the first ~3.4us)
            zb = constp.tile([128, 512], bf16)
            nc.gpsimd.memset(zb[:], 0.0)
            warm_ctr = [0]

            def warm(nmm, rhs=None):
                w = warm_ctr[0]
                warm_ctr[0] += 1
                pw = psump.tile(
                    [O, 512], f32, name=f"ps_warm{w}", tag=f"ps{w % 2}"
                )
                r = zb[:] if rhs is None else rhs
                kp = r.shape[0]
                bp = r.base_partition()
                for j in range(nmm):
                    nc.tensor.matmul(
                        pw[:], zb[bp : bp + kp, 0:128], r,
                        start=(j % 8 == 0), stop=(j % 8 == 7 or j == nmm - 1),
                    )

            warm(4)

            # ---- g = relu(v - 0.5) IN PLACE on the c4 partition groups ----
            # two phases per image: rows [0,34) unblock tiles 0-3, rows
            # [34,64) the rest; each phase split scalar/vector
            def phi(i, phase):
                y = Y[i]
                a, b = (0, 34) if phase == 0 else (34, H)
                m = a + (b - a) * 2 // 5                 # scalar gets ~40%
                for (p0, lo, hi, eng) in (
                    (32, a, m, "s"), (32, m, b, "v"),
                    (96, a, m, "s"), (96, m, b, "v"),
                ):
                    base = 1 + WG if p0 == 32 else 1     # unshifted vs shifted
                    ap = y[p0 : p0 + 32, base + lo * WG : base + hi * WG]
                    if eng == "s":
                        nc.scalar.activation(
                            ap, ap, Act.Relu,
                            bias=neghalf[p0 : p0 + 32], scale=1.0,
                        )
                    else:
                        nc.vector.tensor_scalar(
                            ap, ap, 0.5, 0.0, Alu.subtract, Alu.max,
                        )

            phi(0, 0)
            warm(3, Y[0][32:64, 1 + WG : 1 + WG + 512])
            phi(0, 1)
            warm(2, Y[0][32:64, 1 + WG + 40 * WG : 1 + WG + 40 * WG + 512])
            phi(1, 0)
            phi(1, 1)

            def ywin(y, p0, p1, r0, kw):
                """[p1-p0, RT, W] window: rows r0..r0+RT, cols kw-1..kw-1+W."""
                off = 1 + r0 * WG + (kw - 1)
                return y[p0:p1, off : off + RT * WG].rearrange(
                    "p (r c) -> p r c", r=RT
                )[:, :, 0:W]

            # ---- GEMM: 2 half-batches of 4 tiles per image ----
            def mm_half(i, half, pss):
                y = Y[i]
                ts = range(4 * half, 4 * half + 4)
                for kw in range(KW):
                    for t in ts:
                        nc.tensor.matmul(
                            pss[t][:], wp_sb[:, kw, :],
                            ywin(y, 0, 128, t * RT, kw),
                            start=(kw == 0), stop=False,
                        )
                for t in ts:
                    nc.tensor.matmul(
                        pss[t][:], ws_sb[0:64, 0, :],
                        ywin(y, 0, 64, t * RT + 2, 0),
                        start=False, stop=False,
                    )
                    if ROWTILE:
                        nc.tensor.matmul(
                            pss[t][:], ws_sb[64:128, 0, :],
                            ywin(y, 64, 128, t * RT + 1, 1),
                            start=False, stop=False,
                        )
                    else:
                        nc.tensor.matmul(
                            pss[t][:], ws_sb[0:64, 2, :],
                            ywin(y, 0, 64, t * RT + 2, 1),
                            start=False, stop=False,
                        )
                for t in ts:
                    nc.tensor.matmul(
                        pss[t][:], ws_sb[0:64, 1, :],
                        ywin(y, 0, 64, t * RT + 2, 2),
                        start=False, stop=True,
                    )
                # drains (scalar/vector alternate), then out-DMAs (SP/scalar)
                osbs = {}
                for t in ts:
                    osb = osbp.tile([O, RT * W], bf16, name="osb")
                    osbs[t] = osb
                    if t % 2 == 0:
                        nc.scalar.activation(
                            osb[:], pss[t][:], Act.Identity,
                            bias=b_sb[:, 0:1], scale=1.0,
                        )
                    else:
                        nc.vector.tensor_scalar(
                            osb[:], pss[t][:], b_sb[:, 0:1], None, Alu.add
                        )
                for t in ts:
                    eng = nc.sync if t % 2 == 0 else nc.scalar
                    eng.dma_start(
                        o_d[i, :, t * RT : (t + 1) * RT, :],
                        osbs[t][:].rearrange("o (r w) -> o r w", r=RT),
                    )

            for i in range(IPC):
                pss = [
                    psump.tile([O, RT * W], f32, name=f"ps_i{i}t{t}",
                               tag=f"ps{t}")
                    for t in range(NT)
                ]
                mm_half(i, 0, pss)
                mm_half(i, 1, pss)
    nc.compile()
    return nc


# ---------------------------------------------------------------- entry


def _prep(inputs):
    x = np.ascontiguousarray(np.asarray(inputs["x"], dtype=np.float32))
    weights = np.ascontiguousarray(np.asarray(inputs["weights"], dtype=np.float32))
    bias = np.ascontiguousarray(np.asarray(inputs["bias"], dtype=np.float32))
    positions = np.ascontiguousarray(
        np.asarray(inputs["positions"], dtype=np.float32)
    )
    return x, weights, bias, positions


def _fast_path_ok(x, positions):
    expect = np.linspace(-1.0, 1.0, P, dtype=np.float32)
    return (
        x.shape == (B, C, H, W)
        and positions.shape == (P,)
        and np.array_equal(positions, expect)
        and float(x.min()) >= 0.0
        and float(x.max()) <= 1.0
    )


def kernel(**inputs):
    import ml_dtypes

    x, weights, bias, positions = _prep(inputs)
    if not _fast_path_ok(x, positions):
        return _reference_np(x, weights, bias, positions)

    wpair, wsing, bias_eff, ident_any = _host_weights(weights, bias)
    if ident_any:
        # identity-shortcut weights present: needs the raw-v plane; use the
        # exact fallback rather than a rarely-exercised device path
        return _reference_np(x, weights, bias, positions)

    from concourse.bass_utils import run_bass_kernel_spmd

    nc = _build_nc()
    xg = np.zeros((B, C, H, W + 1), dtype=ml_dtypes.bfloat16)
    xg[:, :, :, :W] = x.astype(ml_dtypes.bfloat16)
    xg = np.ascontiguousarray(xg)
    in_maps = [
        {"xg": xg[i * IPC : (i + 1) * IPC],
         "wpair": wpair, "wsing": wsing, "bias": bias_eff}
        for i in range(NCORES)
    ]
    res = run_bass_kernel_spmd(nc, in_maps, core_ids=list(range(NCORES)))
    out = np.concatenate([res.results[i]["out"] for i in range(NCORES)], axis=0)
    return np.ascontiguousarray(out.astype(np.float32))


# ------------------------------------------------------------ dev utilities


def _run_sim(inputs):
    """CoreSim single-core run (images 0..IPC-1) for correctness debugging."""
    import ml_dtypes
    from concourse.bass_interp import CoreSim

    x, weights, bias, positions = _prep(inputs)
    assert _fast_path_ok(x, positions)
    wpair, wsing, bias_eff, ident_any = _host_weights(weights, bias)
    assert not ident_any
    nc = _build_nc()
    sim = CoreSim(nc)
    xg = np.zeros((IPC, C, H, W + 1), dtype=ml_dtypes.bfloat16)
    xg[:, :, :, :W] = x[:IPC].astype(ml_dtypes.bfloat16)
    sim.tensor("xg")[:] = xg
    sim.tensor("wpair")[:] = wpair
    sim.tensor("wsing")[:] = wsing
    sim.tensor("bias")[:] = bias_eff
    sim.simulate()
    return np.array(sim.tensor("out")).astype(np.float32)
